# revision 1
# baseline (speedup 1.0000x reference)
"""Trainium2 Bass kernel for BaselineFeedforwardNetwork (dense_mlp).

Computation (per path n, step t):
    x_t   = [f_t (3), delta_{t-1} (1)]
    h     = relu(x_t @ W1 + b1)        # 4  -> 64
    h2    = relu(h @ W2 + b2)          # 64 -> 64
    delta = h2 @ W3 + b3               # 64 -> 1
Output: deltas (N, T).

Strategy (8 NeuronCores, pure data parallel over N):
  * hidden dim on SBUF partitions, paths on the free axis
  * per core: 32768 paths, processed as 8 passes of 4096 paths
    (8 chunks x 512); two passes run in lockstep ("lanes") so engines
    stay busy across the serial delta recurrence
  * all matmul operands bf16 (1 cyc/row on PE; fp32 would be 4x slower)
  * one in-place PSUM tile (4 banks) per lane per step: pre1 -> pre2 ->
    deltapre reuse the same banks (Tile serializes via true deps)
  * biases: per-partition bias APs on the activation ops; b3 immediate
  * delta chunk-select tricks: mm3 uses per-chunk lhsT columns so all 8
    chunks' deltas land on contiguous PSUM partitions 0..7; mm1b uses
    per-chunk lhsT rows to consume them from a partition-0-based tile
"""

import sys

for _p in ("/opt/trn_rl_repo",):
    if _p not in sys.path:
        sys.path.insert(0, _p)

import os
import numpy as np
import ml_dtypes

KLVL = int(os.environ.get("KLVL", "4"))  # debug: 1=mm1a/act1 2=+mm2/act2 3=+mm3/act3 4=full

NCORES = 8
N_TOT, T, FDIM = 262144, 60, 3
NC = N_TOT // NCORES          # 32768 paths per core
HID = 64
CH = 512                      # matmul free dim (one PSUM bank of fp32)
G = 8                         # chunks per pass-step
GP = G * CH                   # 4096 paths per pass
NPASS = NC // GP              # 8
NLANES = 2                    # passes in lockstep

# wpack column layout (all bf16, 128 partitions; every block duplicated on
# both partition halves so any chunk parity / lane can read it)
W1A_OFF = 0                                  # rows {0:3, 64:67} = W1[0:3]
M_OFF = 64                                   # rows 0:64 and 64:128 = W3 @ W1[3,:] (rank-1 fold)
W2_OFF = M_OFF + 64                          # rows 0:64 and 64:128 = W2
W3_OFF = W2_OFF + 64                         # [*, W3_OFF+32c+c] = W3 iff select col == c (dup halves)
WCOLS = W3_OFF + 32 * G                      # 448


def _build_graph(npass=NPASS, nsteps=T, b3val=0.0):
    import concourse.bacc as bacc
    from concourse import mybir
    from concourse.tile import TileContext

    BF = mybir.dt.bfloat16
    F32 = mybir.dt.float32

    import time as _time

    nc = bacc.Bacc(trn_type="TRN2", name=f"k{int(_time.time())}")

    feats_p = nc.declare_dram_parameter("feats", [T, FDIM, NC], BF, isOutput=False)
    wpack_p = nc.declare_dram_parameter("wpack", [128, WCOLS], BF, isOutput=False)
    bias_p = nc.declare_dram_parameter("biasp", [128, 4], F32, isOutput=False)
    out_p = nc.declare_dram_parameter("out", [T, NPASS * G, 2 * CH], BF, isOutput=True)

    with TileContext(nc) as tc:
        with (
            tc.tile_pool(name="consts", bufs=1) as cpool,
            tc.tile_pool(name="sbuf", bufs=2) as spool,
            tc.tile_pool(name="xqp", bufs=6) as xpool,
            tc.tile_pool(name="psum", bufs=1, space="PSUM") as ppool,
        ):
            wp = cpool.tile([128, WCOLS], BF, tag="wpack")
            bp = cpool.tile([128, 4], F32, tag="biasp")
            nc.sync.dma_start(out=wp[:, :], in_=wpack_p[:, :])
            nc.sync.dma_start(out=bp[:, :], in_=bias_p[:, :])

            # Warm-up: loads the ACT table + lets ACT/DVE observe const DMAs
            warm = cpool.tile([128, 4], F32, tag="warm")
            nc.scalar.activation(
                warm[:, 0:1], bp[:, 0:1],
                mybir.ActivationFunctionType.Relu, bias=0.0, scale=1.0,
            )
            nc.vector.tensor_scalar(
                warm[:, 1:2], bp[:, 1:2], 0.0, None, mybir.AluOpType.add,
            )

            def dma_x(ln, p, t0):
                xt = xpool.tile([67, GP], BF, tag=f"xq{ln}")
                nc.sync.dma_start(
                    out=xt[0:FDIM, :], in_=feats_p[t0, :, p * GP : (p + 1) * GP]
                )
                nc.sync.dma_start(
                    out=xt[64 : 64 + FDIM, :],
                    in_=feats_p[t0, :, p * GP : (p + 1) * GP],
                )
                return xt

            XPRE = 5  # steps of feature prefetch
            for ppair in range(npass // NLANES):
                lanes = [ppair * NLANES + ln for ln in range(NLANES)]
                xq = [[dma_x(ln, p, t0) for t0 in range(min(XPRE, nsteps))]
                      for ln, p in enumerate(lanes)]
                h2prev = [None] * NLANES
                for t in range(nsteps):
                    for ln, p in enumerate(lanes):
                        if t + XPRE < nsteps:
                            xq[ln].append(dma_x(ln, p, t + XPRE))
                        x = xq[ln][t]
                        P = ppool.tile([128, G // 2 * CH], F32, tag=f"pp{ln}")
                        h = spool.tile([128, G // 2 * CH], BF, tag=f"h{ln}")
                        h2 = spool.tile([128, G // 2 * CH], BF, tag=f"h2{ln}")
                        d_new = spool.tile([8, 2 * CH], BF, tag=f"d{ln}")

                        # lane-dependent partition parities: lane 0 uses the
                        # diagonal PE quadrants, lane 1 the anti-diagonal, so
                        # the two lanes' matmuls run on disjoint subarrays
                        def pH(c):   # pre1 / h partitions
                            return 64 * ((c % 2) ^ ln)

                        def pH2(c):  # pre2 / h2 partitions (and x row copy)
                            return 64 * (c % 2)

                        # ---- layer 1: pre1 = M^T h2prev (+ W1a^T f) ----
                        for c in range(G):
                            blk = (c // 2) * CH
                            o = P[pH(c) : pH(c) + HID, blk : blk + CH]
                            tp = (pH2(c), pH(c))
                            if t > 0:
                                nc.tensor.matmul(
                                    o,
                                    wp[pH2(c) : pH2(c) + HID, M_OFF : M_OFF + HID],
                                    h2prev[ln][pH2(c) : pH2(c) + HID, blk : blk + CH],
                                    start=True,
                                    stop=False,
                                    tile_position=tp,
                                )
                            nc.tensor.matmul(
                                o,
                                wp[pH2(c) : pH2(c) + FDIM, W1A_OFF : W1A_OFF + HID],
                                x[pH2(c) : pH2(c) + FDIM, c * CH : (c + 1) * CH],
                                start=(t == 0),
                                stop=True,
                                tile_position=tp,
                            )
                        # ---- act1: h = relu(pre1 + b1') ----
                        # t=0 uses plain b1 (no delta yet); t>0 uses
                        # b1' = b1 + W1[3,:]*b3 (completes the rank-1 fold)
                        nc.scalar.activation(
                            h[:, :], P[:, :],
                            mybir.ActivationFunctionType.Relu,
                            bias=bp[:, 3:4] if t == 0 else bp[:, 0:1],
                            scale=1.0,
                        )
                        # ---- layer 2 ----
                        for c in range(G):
                            blk = (c // 2) * CH
                            nc.tensor.matmul(
                                P[pH2(c) : pH2(c) + HID, blk : blk + CH],
                                wp[pH(c) : pH(c) + HID, W2_OFF : W2_OFF + HID],
                                h[pH(c) : pH(c) + HID, blk : blk + CH],
                                start=True,
                                stop=True,
                                tile_position=(pH(c), pH2(c)),
                            )
                        # ---- act2: h2 = relu(pre2 + b2) on DVE ----
                        nc.vector.tensor_scalar(
                            h2[:, :], P[:, :],
                            bp[:, 1:2], 0.0,
                            mybir.AluOpType.add, mybir.AluOpType.max,
                        )
                        # ---- layer 3 select: chunk deltas -> PSUM rows ----
                        # lane0 rows 0:8, lane1 rows 32:40; even chunks into
                        # cols 0:CH, odd into CH:2CH; uniform positions per group
                        dr = 32 * ln
                        for par in range(2):
                            cs = [c for c in range(G) if c % 2 == par]
                            for i, c in enumerate(cs):
                                blk = (c // 2) * CH
                                nc.tensor.matmul(
                                    P[dr : dr + 32, par * CH : (par + 1) * CH],
                                    wp[pH2(c) : pH2(c) + HID, W3_OFF + 32 * c : W3_OFF + 32 * (c + 1)],
                                    h2[pH2(c) : pH2(c) + HID, blk : blk + CH],
                                    start=(i == 0),
                                    stop=(i == len(cs) - 1),
                                    tile_position=(pH2(c), dr),
                                )
                        # ---- act3: delta = deltapre + b3 ----
                        nc.scalar.activation(
                            d_new[0:G, :], P[dr : dr + G, 0 : 2 * CH],
                            mybir.ActivationFunctionType.Copy,
                            bias=float(b3val), scale=1.0,
                        )
                        # deltas out: both col-halves; host selects by parity
                        nc.sync.dma_start(
                            out=out_p[t, p * G : (p + 1) * G, :],
                            in_=d_new[0:G, :],
                        )
                        h2prev[ln] = h2
    return nc


LAST_RESULT = None


def kernel(**inputs):
    return _run(inputs, NPASS, T)


def _prepare(inputs, npass, nsteps):
    features = np.asarray(inputs["features"], dtype=np.float32)
    W1 = np.asarray(inputs["W1"], dtype=np.float32)
    b1 = np.asarray(inputs["b1"], dtype=np.float32)
    W2 = np.asarray(inputs["W2"], dtype=np.float32)
    b2 = np.asarray(inputs["b2"], dtype=np.float32)
    W3 = np.asarray(inputs["W3"], dtype=np.float32)
    b3 = np.asarray(inputs["b3"], dtype=np.float32)

    nc = _build_graph(npass, nsteps, float(b3[0]))
    nc.finalize()

    # host-side packing
    bf = ml_dtypes.bfloat16
    wpack = np.zeros((128, WCOLS), np.float32)
    M = W3 @ W1[3:4]  # (64, 64) rank-1: M[i, j] = W3[i] * W1[3, j]
    for half in (0, 64):
        wpack[half : half + 3, W1A_OFF : W1A_OFF + HID] = W1[0:3]
        wpack[half : half + HID, M_OFF : M_OFF + HID] = M
        wpack[half : half + HID, W2_OFF : W2_OFF + HID] = W2
        for c in range(G):
            wpack[half : half + HID, W3_OFF + 32 * c + c] = W3[:, 0]
    wpack = wpack.astype(bf)

    b1p = b1 + W1[3] * b3[0]
    biasp = np.zeros((128, 4), np.float32)
    for half in (0, 64):
        biasp[half : half + HID, 0] = b1p
        biasp[half : half + HID, 1] = b2
        biasp[half : half + HID, 3] = b1
    biasp[:, 2] = b3[0]

    in_maps = []
    for k in range(NCORES):
        sh = features[k * NC : (k + 1) * NC]          # (NC, T, 3)
        feats = np.ascontiguousarray(sh.transpose(1, 2, 0)).astype(bf)  # (T,3,NC)
        in_maps.append({"feats": feats, "wpack": wpack, "biasp": biasp})

    return nc, in_maps


def _run(inputs, npass, nsteps, trace=False):
    global LAST_RESULT
    from concourse.bass_utils import run_bass_kernel_spmd

    nc, in_maps = _prepare(inputs, npass, nsteps)
    res = run_bass_kernel_spmd(
        nc, in_maps, core_ids=list(range(NCORES)), trace=trace
    )
    LAST_RESULT = res
    outs = res.results

    full = np.empty((N_TOT, T), np.float32)
    rows = np.arange(NPASS * G)
    par = rows % 2
    for k in range(NCORES):
        o = np.asarray(outs[k]["out"]).astype(np.float32)  # (T, 64, 2*CH)
        o = o.reshape(T, NPASS * G, 2, CH)[:, rows, par, :]  # (T, 64, CH)
        full[k * NC : (k + 1) * NC, :] = o.reshape(T, NC).T
    return full


if __name__ == "__main__":
    import reference

    inputs = reference.setup_inputs()
    out = kernel(**{k: np.asarray(v) for k, v in inputs.items()})
    print("kernel out", out.shape, out.dtype)



# revision 5
# speedup vs baseline: 2.2464x; 2.2464x over previous
"""Trainium2 Bass kernel for BaselineFeedforwardNetwork (dense_mlp).

Computation (per path n, step t):
    x_t   = [f_t (3), delta_{t-1} (1)]
    h     = relu(x_t @ W1 + b1)        # 4  -> 64
    h2    = relu(h @ W2 + b2)          # 64 -> 64
    delta = h2 @ W3 + b3               # 64 -> 1
Output: deltas (N, T).

Strategy (8 NeuronCores, pure data parallel over N):
  * 32768 paths/core; "pair" = 1024 paths = 2 chunks of 512 stacked on
    the 128 partitions (chunk A -> hidden rows 0:64, chunk B -> 64:128)
  * all matmuls are full-width K=128 (or K<=8 for layer 1) with
    block-diagonal weights -> 3 matmuls x 512 columns per pair-step,
    the minimum PE streaming for this net (PE issue is serial on TRN2)
  * delta feedback is direct: layer-3 deltas for 8 pairs accumulate
    into one [16,512] PSUM tile via column-select lhsT, ACT copies
    them to SBUF (bf16, +b3), small SBUF->SBUF DMAs scatter them into
    the delta rows of the NEXT step's x tile; block ordering hides the
    recurrence latency behind the other blocks' compute
  * x tile per step: [128, 4096] bf16; partition group 32g holds rows
    [fA(3), fB(3), dA, dB] for pairs p=4q+g at free cols 512q
  * features pre-packed on host to [T, 4, 6, 4096] so each step's
    feature load is one large contiguous DMA
"""

import sys

for _p in ("/opt/trn_rl_repo",):
    if _p not in sys.path:
        sys.path.insert(0, _p)

import numpy as np
import ml_dtypes

NCORES = 8
N_TOT, T, FDIM = 262144, 60, 3
NC = N_TOT // NCORES          # 32768 paths per core
HID = 64
CH = 512                      # matmul free dim (one PSUM bank of fp32)
NPAIR = 32                    # pairs (1024 paths) per core-step
NBLK = 4                      # layer-3 blocks of 8 pairs
XPRE = 4                      # steps of feature prefetch

# wpack column layout (bf16, 128 partitions)
W1_OFF = 0                    # cols 0:128, rows 32g+{0..7}: L1 lhsT per group
W2_OFF = 128                  # cols 128:256: blockdiag(W2, W2)
W3_OFF = 256                  # cols 256:384: 8 x [128,16] L3 select lhsT
WCOLS = 384


def _build_graph(nsteps=T, b3val=0.0):
    import concourse.bacc as bacc
    from concourse import mybir
    from concourse.tile import TileContext

    BF = mybir.dt.bfloat16
    F32 = mybir.dt.float32

    import time as _time

    nc = bacc.Bacc(trn_type="TRN2", name=f"k{int(_time.time())}")

    feats_p = nc.declare_dram_parameter("feats", [nsteps, 4, 6, 8 * CH], BF,
                                        isOutput=False)
    wpack_p = nc.declare_dram_parameter("wpack", [128, WCOLS], BF,
                                        isOutput=False)
    bias_p = nc.declare_dram_parameter("biasp", [128, 2], F32, isOutput=False)
    out_p = nc.declare_dram_parameter("out", [nsteps, 16, NBLK * CH], BF,
                                      isOutput=True)

    with TileContext(nc) as tc:
        with (
            tc.tile_pool(name="consts", bufs=1) as cpool,
            tc.tile_pool(name="xq", bufs=XPRE + 2) as xpool,
            tc.tile_pool(name="hh", bufs=3) as hpool,
            tc.tile_pool(name="dst", bufs=2) as dpool,
            tc.tile_pool(name="ps", bufs=3, space="PSUM") as ppool,
            tc.tile_pool(name="ps3", bufs=2, space="PSUM") as ppool3,
        ):
            wp = cpool.tile([128, WCOLS], BF, tag="wpack")
            bp = cpool.tile([128, 2], F32, tag="biasp")
            nc.sync.dma_start(out=wp[:, :], in_=wpack_p[:, :])
            nc.sync.dma_start(out=bp[:, :], in_=bias_p[:, :])

            # Warm-up: loads the ACT table + lets ACT/DVE observe const DMAs
            warm = cpool.tile([128, 4], F32, tag="warm")
            nc.scalar.activation(
                warm[:, 0:1], bp[:, 0:1],
                mybir.ActivationFunctionType.Relu, bias=0.0, scale=1.0,
            )
            nc.vector.tensor_scalar(
                warm[:, 1:2], bp[:, 1:2], 0.0, None, mybir.AluOpType.add,
            )

            def dma_x(t0):
                xt = xpool.tile([128, 8 * CH], BF, tag="x")
                for g in range(4):
                    nc.sync.dma_start(
                        out=xt[32 * g: 32 * g + 6, :], in_=feats_p[t0, g]
                    )
                return xt

            xq = [dma_x(t0) for t0 in range(min(XPRE, nsteps))]

            for t in range(nsteps):
                if t + XPRE < nsteps:
                    xq.append(dma_x(t + XPRE))
                xt = xq[t]
                xnext = xq[t + 1] if t + 1 < nsteps else None
                dstage = dpool.tile([16, NBLK * CH], BF, tag="dstage")

                for b in range(NBLK):
                    acc3 = ppool3.tile([16, CH], F32, tag="l3acc")
                    for qq in range(2):
                        q = 2 * b + qq
                        for g in range(4):
                            i = 4 * qq + g  # pair index within block
                            P1 = ppool.tile([128, CH], F32, tag="pre1")
                            P2 = ppool.tile([128, CH], F32, tag="pre2")
                            h = hpool.tile([128, CH], BF, tag="h")
                            h2 = hpool.tile([128, CH], BF, tag="h2")

                            # ---- layer 1 (K=8; t=0 uses K=6: no delta) ----
                            K1 = 6 if t == 0 else 8
                            nc.tensor.matmul(
                                P1[:, :],
                                wp[32 * g: 32 * g + K1, W1_OFF: W1_OFF + 128],
                                xt[32 * g: 32 * g + K1,
                                   CH * q: CH * (q + 1)],
                                start=True, stop=True,
                                tile_position=(32 * g, 0),
                            )
                            # ---- act1: h = relu(pre1 + b1) on ACT ----
                            nc.scalar.activation(
                                h[:, :], P1[:, :],
                                mybir.ActivationFunctionType.Relu,
                                bias=bp[:, 0:1], scale=1.0,
                            )
                            # ---- layer 2 (K=128 blockdiag W2) ----
                            nc.tensor.matmul(
                                P2[:, :],
                                wp[:, W2_OFF: W2_OFF + 128],
                                h[:, :],
                                start=True, stop=True,
                            )
                            # ---- act2: h2 = relu(pre2 + b2) on DVE ----
                            nc.vector.tensor_scalar(
                                h2[:, :], P2[:, :],
                                bp[:, 1:2], 0.0,
                                mybir.AluOpType.add, mybir.AluOpType.max,
                            )
                            # ---- layer 3: select-accumulate into acc3 ----
                            nc.tensor.matmul(
                                acc3[0:16, :],
                                wp[:, W3_OFF + 16 * i: W3_OFF + 16 * (i + 1)],
                                h2[:, :],
                                start=(i == 0), stop=(i == 7),
                            )
                    # ---- act3: dstage block = acc3 + b3 (bf16) ----
                    nc.scalar.activation(
                        dstage[0:16, CH * b: CH * (b + 1)], acc3[0:16, :],
                        mybir.ActivationFunctionType.Copy,
                        bias=float(b3val), scale=1.0,
                    )
                    # ---- scatter deltas into next step's x tile ----
                    if xnext is not None:
                        for qq in range(2):
                            q = 2 * b + qq
                            for c in range(2):
                                nc.sync.dma_start(
                                    out=xnext.rearrange(
                                        "(g r) n -> g r n", g=4
                                    )[:, 6 + c, CH * q: CH * (q + 1)],
                                    in_=dstage[8 * qq: 8 * qq + 8,
                                               CH * b: CH * (b + 1)].rearrange(
                                        "(g c) n -> c g n", g=4
                                    )[c],
                                )
                # ---- deltas out ----
                nc.sync.dma_start(out=out_p[t], in_=dstage[0:16, :])
    return nc


LAST_RESULT = None


def kernel(**inputs):
    return _run(inputs, T)


def _prepare(inputs, nsteps):
    features = np.asarray(inputs["features"], dtype=np.float32)
    W1 = np.asarray(inputs["W1"], dtype=np.float32)
    b1 = np.asarray(inputs["b1"], dtype=np.float32)
    W2 = np.asarray(inputs["W2"], dtype=np.float32)
    b2 = np.asarray(inputs["b2"], dtype=np.float32)
    W3 = np.asarray(inputs["W3"], dtype=np.float32)
    b3 = np.asarray(inputs["b3"], dtype=np.float32)

    nc = _build_graph(nsteps, float(b3[0]))
    nc.finalize()

    bf = ml_dtypes.bfloat16

    # wpack
    wpack = np.zeros((128, WCOLS), np.float32)
    for g in range(4):
        for c in range(2):
            for f in range(FDIM):
                wpack[32 * g + 3 * c + f,
                      W1_OFF + 64 * c: W1_OFF + 64 * (c + 1)] = W1[f]
            wpack[32 * g + 6 + c,
                  W1_OFF + 64 * c: W1_OFF + 64 * (c + 1)] = W1[3]
    for c in range(2):
        wpack[64 * c: 64 * (c + 1),
              W2_OFF + 64 * c: W2_OFF + 64 * (c + 1)] = W2
    for i in range(8):  # i = 4*qq + g
        qq, g = divmod(i, 4)
        for c in range(2):
            m = 8 * qq + 2 * g + c
            wpack[64 * c: 64 * (c + 1), W3_OFF + 16 * i + m] = W3[:, 0]
    wpack = wpack.astype(bf)

    biasp = np.zeros((128, 2), np.float32)
    for half in (0, 64):
        biasp[half: half + HID, 0] = b1
        biasp[half: half + HID, 1] = b2

    # features: path = 4096 q + 1024 g + 512 c + j
    # host layout F[t, g, 3c+f, 512 q + j] = features[path, t, f]
    in_maps = []
    for k in range(NCORES):
        sh = features[k * NC: (k + 1) * NC, :nsteps, :]   # (NC, ns, 3)
        v = sh.reshape(8, 4, 2, CH, nsteps, FDIM)          # q g c j t f
        v = v.transpose(4, 1, 2, 5, 0, 3)                  # t g c f q j
        feats = np.ascontiguousarray(v).reshape(
            nsteps, 4, 6, 8 * CH).astype(bf)
        in_maps.append({"feats": feats, "wpack": wpack, "biasp": biasp})

    return nc, in_maps


def _unscramble(o, nsteps):
    # o: (ns, 16, 2048) bf16; rows r = 8 qq + 2 g + c, cols = 512 b + j
    # path = 4096 (2b + qq) + 1024 g + 512 c + j
    v = np.asarray(o).astype(np.float32)
    v = v.reshape(nsteps, 2, 4, 2, NBLK, CH)        # t qq g c b j
    v = v.transpose(4, 1, 2, 3, 5, 0)               # b qq g c j t
    return v.reshape(NC, nsteps)


def _run(inputs, nsteps, trace=False):
    global LAST_RESULT
    from concourse.bass_utils import run_bass_kernel_spmd

    nc, in_maps = _prepare(inputs, nsteps)
    res = run_bass_kernel_spmd(
        nc, in_maps, core_ids=list(range(NCORES)), trace=trace
    )
    LAST_RESULT = res
    outs = res.results

    full = np.empty((N_TOT, nsteps), np.float32)
    for k in range(NCORES):
        full[k * NC: (k + 1) * NC] = _unscramble(outs[k]["out"], nsteps)
    return full


if __name__ == "__main__":
    import reference

    inputs = reference.setup_inputs()
    out = kernel(**{k: np.asarray(v) for k, v in inputs.items()})
    print("kernel out", out.shape, out.dtype)


# revision 6
# speedup vs baseline: 3.2796x; 1.4600x over previous
"""Trainium2 Bass kernel for BaselineFeedforwardNetwork (dense_mlp).

Computation (per path n, step t):
    x_t   = [f_t (3), delta_{t-1} (1)]
    h     = relu(x_t @ W1 + b1)        # 4  -> 64
    h2    = relu(h @ W2 + b2)          # 64 -> 64
    delta = h2 @ W3 + b3               # 64 -> 1
Output: deltas (N, T).

Strategy (8 NeuronCores, pure data parallel over N):
  * 32768 paths/core; "pair" = 1024 paths = 2 chunks of 512 stacked on
    the 128 partitions (chunk A -> hidden rows 0:64, chunk B -> 64:128)
  * all matmuls are full-width K=128 (or K<=8 for layer 1) with
    block-diagonal weights -> 3 matmuls x 512 columns per pair-step,
    the minimum PE streaming for this net (PE issue is serial on TRN2)
  * delta feedback is direct: layer-3 deltas for 8 pairs accumulate
    into one [16,512] PSUM tile via column-select lhsT, ACT copies
    them to SBUF (bf16, +b3), small SBUF->SBUF DMAs scatter them into
    the delta rows of the NEXT step's x tile; block ordering hides the
    recurrence latency behind the other blocks' compute
  * x tile per step: [128, 4096] bf16; partition group 32g holds rows
    [fA(3), fB(3), dA, dB] for pairs p=4q+g at free cols 512q
  * features pre-packed on host to [T, 4, 6, 4096] so each step's
    feature load is one large contiguous DMA
"""

import sys

for _p in ("/opt/trn_rl_repo",):
    if _p not in sys.path:
        sys.path.insert(0, _p)

import numpy as np
import ml_dtypes

NCORES = 8
N_TOT, T, FDIM = 262144, 60, 3
NC = N_TOT // NCORES          # 32768 paths per core
HID = 64
CH = 512                      # matmul free dim (one PSUM bank of fp32)
NPAIR = 32                    # pairs (1024 paths) per core-step
NBLK = 4                      # layer-3 blocks of 8 pairs
XPRE = 4                      # steps of feature prefetch

# wpack column layout (bf16, 128 partitions)
W1_OFF = 0                    # cols 0:128, rows 32g+{0..7}: L1 lhsT per group
W2_OFF = 128                  # cols 128:256: blockdiag(W2, W2)
W3_OFF = 256                  # cols 256:384: 8 x [128,16] L3 select lhsT
WCOLS = 384


def _build_graph(nsteps=T, b3val=0.0):
    import concourse.bacc as bacc
    from concourse import mybir
    from concourse.tile import TileContext

    BF = mybir.dt.bfloat16
    F32 = mybir.dt.float32

    import time as _time

    nc = bacc.Bacc(trn_type="TRN2", name=f"k{int(_time.time())}")

    feats_p = nc.declare_dram_parameter("feats", [nsteps, 4, 6, 8 * CH], BF,
                                        isOutput=False)
    wpack_p = nc.declare_dram_parameter("wpack", [128, WCOLS], BF,
                                        isOutput=False)
    bias_p = nc.declare_dram_parameter("biasp", [128, 2], F32, isOutput=False)
    out_p = nc.declare_dram_parameter("out", [nsteps, 16, NBLK * CH], BF,
                                      isOutput=True)

    with TileContext(nc) as tc:
        with (
            tc.tile_pool(name="consts", bufs=1) as cpool,
            tc.tile_pool(name="xq", bufs=XPRE + 2) as xpool,
            tc.tile_pool(name="hh", bufs=3) as hpool,
            tc.tile_pool(name="dst", bufs=2) as dpool,
            tc.tile_pool(name="ps", bufs=3, space="PSUM") as ppool,
            tc.tile_pool(name="ps3", bufs=2, space="PSUM") as ppool3,
        ):
            wp = cpool.tile([128, WCOLS], BF, tag="wpack")
            bp = cpool.tile([128, 2], F32, tag="biasp")
            nc.sync.dma_start(out=wp[:, :], in_=wpack_p[:, :])
            nc.sync.dma_start(out=bp[:, :], in_=bias_p[:, :])

            # Warm-up: loads the ACT table + lets ACT/DVE observe const DMAs
            warm = cpool.tile([128, 4], F32, tag="warm")
            nc.scalar.activation(
                warm[:, 0:1], bp[:, 0:1],
                mybir.ActivationFunctionType.Relu, bias=0.0, scale=1.0,
            )
            nc.vector.tensor_scalar(
                warm[:, 1:2], bp[:, 1:2], 0.0, None, mybir.AluOpType.add,
            )

            def dma_x(t0):
                xt = xpool.tile([128, 8 * CH], BF, tag="x")
                for g in range(4):
                    nc.sync.dma_start(
                        out=xt[32 * g: 32 * g + 6, :], in_=feats_p[t0, g]
                    )
                return xt

            xq = [dma_x(t0) for t0 in range(min(XPRE, nsteps))]

            for t in range(nsteps):
                if t + XPRE < nsteps:
                    xq.append(dma_x(t + XPRE))
                xt = xq[t]
                xnext = xq[t + 1] if t + 1 < nsteps else None
                dstage = dpool.tile([16, NBLK * CH], BF, tag="dstage")

                for b in range(NBLK):
                    acc3 = ppool3.tile([16, CH], F32, tag="l3acc")
                    for g in range(4):
                        # duo: pairs (q=2b, g) and (q=2b+1, g)
                        P = ppool.tile([128, 2 * CH], F32, tag="pre")
                        h = hpool.tile([128, 2 * CH], BF, tag="h")
                        h2 = hpool.tile([128, 2 * CH], BF, tag="h2")

                        # ---- layer 1 (K=8; t=0 uses K=6: no delta) ----
                        K1 = 6 if t == 0 else 8
                        for qq in range(2):
                            q = 2 * b + qq
                            nc.tensor.matmul(
                                P[:, CH * qq: CH * (qq + 1)],
                                wp[32 * g: 32 * g + K1, W1_OFF: W1_OFF + 128],
                                xt[32 * g: 32 * g + K1,
                                   CH * q: CH * (q + 1)],
                                start=True, stop=True,
                                tile_position=(32 * g, 0),
                            )
                        # ---- act1: h = relu(pre1 + b1) on ACT ----
                        nc.scalar.activation(
                            h[:, :], P[:, :],
                            mybir.ActivationFunctionType.Relu,
                            bias=bp[:, 0:1], scale=1.0,
                        )
                        # ---- layer 2 (K=128 blockdiag W2, in-place) ----
                        for qq in range(2):
                            nc.tensor.matmul(
                                P[:, CH * qq: CH * (qq + 1)],
                                wp[:, W2_OFF: W2_OFF + 128],
                                h[:, CH * qq: CH * (qq + 1)],
                                start=True, stop=True,
                            )
                        # ---- act2: h2 = relu(pre2 + b2) on DVE ----
                        nc.vector.tensor_scalar(
                            h2[:, :], P[:, :],
                            bp[:, 1:2], 0.0,
                            mybir.AluOpType.add, mybir.AluOpType.max,
                        )
                        # ---- layer 3: select-accumulate into acc3 ----
                        for qq in range(2):
                            i = 4 * qq + g
                            nc.tensor.matmul(
                                acc3[0:16, :],
                                wp[:, W3_OFF + 16 * i: W3_OFF + 16 * (i + 1)],
                                h2[:, CH * qq: CH * (qq + 1)],
                                start=(g == 0 and qq == 0),
                                stop=(g == 3 and qq == 1),
                            )
                    # ---- act3: dstage block = acc3 + b3 (bf16) ----
                    nc.scalar.activation(
                        dstage[0:16, CH * b: CH * (b + 1)], acc3[0:16, :],
                        mybir.ActivationFunctionType.Copy,
                        bias=float(b3val), scale=1.0,
                    )
                    # ---- scatter deltas into next step's x tile ----
                    if xnext is not None:
                        for qq in range(2):
                            q = 2 * b + qq
                            for c in range(2):
                                nc.sync.dma_start(
                                    out=xnext.rearrange(
                                        "(g r) n -> g r n", g=4
                                    )[:, 6 + c, CH * q: CH * (q + 1)],
                                    in_=dstage[8 * qq: 8 * qq + 8,
                                               CH * b: CH * (b + 1)].rearrange(
                                        "(g c) n -> c g n", g=4
                                    )[c],
                                )
                # ---- deltas out ----
                nc.sync.dma_start(out=out_p[t], in_=dstage[0:16, :])
    return nc


LAST_RESULT = None


def kernel(**inputs):
    return _run(inputs, T)


def _prepare(inputs, nsteps):
    features = np.asarray(inputs["features"], dtype=np.float32)
    W1 = np.asarray(inputs["W1"], dtype=np.float32)
    b1 = np.asarray(inputs["b1"], dtype=np.float32)
    W2 = np.asarray(inputs["W2"], dtype=np.float32)
    b2 = np.asarray(inputs["b2"], dtype=np.float32)
    W3 = np.asarray(inputs["W3"], dtype=np.float32)
    b3 = np.asarray(inputs["b3"], dtype=np.float32)

    nc = _build_graph(nsteps, float(b3[0]))
    nc.finalize()

    bf = ml_dtypes.bfloat16

    # wpack
    wpack = np.zeros((128, WCOLS), np.float32)
    for g in range(4):
        for c in range(2):
            for f in range(FDIM):
                wpack[32 * g + 3 * c + f,
                      W1_OFF + 64 * c: W1_OFF + 64 * (c + 1)] = W1[f]
            wpack[32 * g + 6 + c,
                  W1_OFF + 64 * c: W1_OFF + 64 * (c + 1)] = W1[3]
    for c in range(2):
        wpack[64 * c: 64 * (c + 1),
              W2_OFF + 64 * c: W2_OFF + 64 * (c + 1)] = W2
    for i in range(8):  # i = 4*qq + g
        qq, g = divmod(i, 4)
        for c in range(2):
            m = 8 * qq + 2 * g + c
            wpack[64 * c: 64 * (c + 1), W3_OFF + 16 * i + m] = W3[:, 0]
    wpack = wpack.astype(bf)

    biasp = np.zeros((128, 2), np.float32)
    for half in (0, 64):
        biasp[half: half + HID, 0] = b1
        biasp[half: half + HID, 1] = b2

    # features: path = 4096 q + 1024 g + 512 c + j
    # host layout F[t, g, 3c+f, 512 q + j] = features[path, t, f]
    in_maps = []
    for k in range(NCORES):
        sh = features[k * NC: (k + 1) * NC, :nsteps, :]   # (NC, ns, 3)
        v = sh.reshape(8, 4, 2, CH, nsteps, FDIM)          # q g c j t f
        v = v.transpose(4, 1, 2, 5, 0, 3)                  # t g c f q j
        feats = np.ascontiguousarray(v).reshape(
            nsteps, 4, 6, 8 * CH).astype(bf)
        in_maps.append({"feats": feats, "wpack": wpack, "biasp": biasp})

    return nc, in_maps


def _unscramble(o, nsteps):
    # o: (ns, 16, 2048) bf16; rows r = 8 qq + 2 g + c, cols = 512 b + j
    # path = 4096 (2b + qq) + 1024 g + 512 c + j
    v = np.asarray(o).astype(np.float32)
    v = v.reshape(nsteps, 2, 4, 2, NBLK, CH)        # t qq g c b j
    v = v.transpose(4, 1, 2, 3, 5, 0)               # b qq g c j t
    return v.reshape(NC, nsteps)


def _run(inputs, nsteps, trace=False):
    global LAST_RESULT
    from concourse.bass_utils import run_bass_kernel_spmd

    nc, in_maps = _prepare(inputs, nsteps)
    res = run_bass_kernel_spmd(
        nc, in_maps, core_ids=list(range(NCORES)), trace=trace
    )
    LAST_RESULT = res
    outs = res.results

    full = np.empty((N_TOT, nsteps), np.float32)
    for k in range(NCORES):
        full[k * NC: (k + 1) * NC] = _unscramble(outs[k]["out"], nsteps)
    return full


if __name__ == "__main__":
    import reference

    inputs = reference.setup_inputs()
    out = kernel(**{k: np.asarray(v) for k, v in inputs.items()})
    print("kernel out", out.shape, out.dtype)
